# revision 24
# baseline (speedup 1.0000x reference)
"""GroupedQueryAttention TRN2 kernel (v2).

Sharding: 8 cores = (batch b in 0..1) x (kv-group g in 0..3). Each core
computes, for its batch and its kv head group (1 kv head, 4 query heads):
  q = x[b] @ Wq[:, g*256:(g+1)*256]          [2048, 256]
  k = x[b] @ Wkv[:, g*64:(g+1)*64]           [2048, 64]
  v = x[b] @ Wkv[:, 256+g*64:256+(g+1)*64]   [2048, 64]
  causal softmax attention per head          [2048, 256]
  partial_out = attn_out @ Wo[g*256:(g+1)*256, :]   [2048, 1024]
Host sums the 4 partials per batch (row-parallel Wo).

Key performance structure (vs v1):
  - Scores (K=64) for the two heads of a pair run CONCURRENTLY on the PE
    via distinct 64-row groups: head-lo keys/queries live on partitions
    0:64 (kva), head-hi on 64:128 (kvb dup).  ~2x on score matmuls.
  - attn@v is split into two K=64 halves (keys 0:64 / 64:128 of each key
    tile) on the two row groups, accumulating into separate PSUM banks;
    the halves are merged by the DVE tensor_tensor add that also serves
    as the PSUM->SBUF drain.  ~2x on av matmuls.
  - kv projection computes [kT; vT] once (single 128-col m-tile); the
    odd-head kT dup at partitions 64:128 comes from a cheap SBUF->SBUF
    DMA bounce instead of a second matmul.
  - exp runs as ONE ACT instruction per key tile covering both paired
    heads ([128, 2, w] across two PSUM banks).
  - Projections (qkv, out) stay fp32r (full PE speed at N=512);
    everything in the attention core is bf16.  Stage transposes carry
    the softmax denominator row inside the [65,128] slab transposes.
  - Partial outputs are written bf16 (halves output DMA); host sums in
    fp64.
"""

import numpy as np
import ml_dtypes

import concourse.bass as bass
import concourse.mybir as mybir
import concourse.tile as tile
from concourse import bacc
from concourse.bass_utils import run_bass_kernel_spmd

B, T, DIM = 2, 2048, 1024
NH, NKV = 16, 4
HD = DIM // NH  # 64
R = NH // NKV  # 4
HQ = R * HD  # 256 query cols per core
NJ = T // 128  # 16 key tiles
NCH = T // 512  # 4 query chunks of 512
NEG = -30000.0

F32R = mybir.dt.float32r
BF16 = mybir.dt.bfloat16
F32 = mybir.dt.float32

_CACHED_NC = None


def _cfg(c, j):
    """Per (chunk, key-tile): (start within chunk, width, has_diag_mask)."""
    m = j - 4 * c
    if m < 0:
        return 0, 512, False
    return 128 * m, 512 - 128 * m, True


def build_nc():
    nc = bacc.Bacc()
    # host-packed layouts: one contiguous DMA each (SP descriptor-gen is
    # ~1.5us per dma_start, so instruction count matters more than bytes)
    xp = nc.declare_dram_parameter("xp", [128, NCH * 8 * 512], BF16, isOutput=False)
    w1 = nc.declare_dram_parameter("w1", [128, 3072], BF16, isOutput=False)
    w2 = nc.declare_dram_parameter("w2", [128, 2304], BF16, isOutput=False)
    out = nc.declare_dram_parameter("out", [T, DIM], BF16, isOutput=True)

    with tile.TileContext(nc) as tc:
        with (
            tc.tile_pool(name="persist", bufs=1) as pp,
            tc.tile_pool(name="vaug_p", bufs=NJ) as vp,
            tc.tile_pool(name="ptt_p", bufs=3) as ptp,
            tc.tile_pool(name="avd_p", bufs=2) as adp,
            tc.tile_pool(name="avs_p", bufs=2) as avsp,
            tc.tile_pool(name="rt_p", bufs=2) as rtp,
            tc.tile_pool(name="avh_p", bufs=2) as ahp,
            tc.tile_pool(name="out_p", bufs=3) as op,
            tc.tile_pool(name="ps_s", bufs=2, space="PSUM") as pss,
            tc.tile_pool(name="ps_av", bufs=1, space="PSUM") as psav,
            tc.tile_pool(name="ps_m", bufs=2, space="PSUM") as psm,
        ):
            # ---- constants / weights (big contiguous DMAs) ----
            xt_sb = pp.tile([128, NCH, 8, 512], BF16, tag="xt")
            nc.sync.dma_start(out=xt_sb[:, 0, 0:4, :], in_=xp[:, 0:2048])
            w1_sb = pp.tile([128, 3072], BF16, tag="w1")
            nc.sync.dma_start(out=w1_sb, in_=w1[:, :])
            nc.sync.dma_start(out=xt_sb[:, 0, 4:8, :], in_=xp[:, 2048:4096])
            w2_sb = pp.tile([128, 2304], BF16, tag="w2")
            nc.sync.dma_start(out=w2_sb, in_=w2[:, :])
            for n in range(1, NCH):
                nc.sync.dma_start(
                    out=xt_sb[:, n, :, :], in_=xp[:, n * 4096 : (n + 1) * 4096]
                )

            def wq_ap(kd, lo, hi):  # wq columns lo:hi of contraction row kd
                return w1_sb[:, kd * HQ + lo : kd * HQ + hi]

            def wkv_ap(kd):
                return w1_sb[:, 2048 + kd * 128 : 2048 + (kd + 1) * 128]

            ident_b = w2_sb[:, 0:128]
            mlt_sb = w2_sb[:, 128:256]

            def wo_ap(cpair, lo, hi):
                return w2_sb[:, 256 + cpair * DIM + lo : 256 + cpair * DIM + hi]

            # attention-core persistent state (all bf16)
            qt_sb = pp.tile([128, 2, T], BF16, tag="qt")  # head h: part (h%2)*64
            kva_sb = pp.tile([128, T], BF16, tag="kva")  # rows 0:64  = kT (lo)
            kvb_sb = pp.tile([128, T], BF16, tag="kvb")  # rows 64:128 = kT (hi)
            vtb_sb = pp.tile([128, T], BF16, tag="vtb")  # rows 64:128 = vT
            avt01 = pp.tile([128, T], BF16, tag="avt01")
            avt23 = pp.tile([128, T], BF16, tag="avt23")

            vaug = [None] * NJ

            # ---- qkv projection pieces for chunk n ----
            def q_mtile(n, m):
                def run():
                    cols = slice(n * 512, (n + 1) * 512)
                    pq = psm.tile([128, 512], F32, tag="m")
                    for kd in range(8):
                        nc.tensor.matmul(
                            pq,
                            lhsT=wq_ap(kd, m * 128, (m + 1) * 128),
                            rhs=xt_sb[:, n, kd, :],
                            start=(kd == 0),
                            stop=(kd == 7),
                        )
                    if n == 0:
                        nc.scalar.copy(out=qt_sb[:, m, cols], in_=pq)
                    else:
                        nc.vector.tensor_copy(out=qt_sb[:, m, cols], in_=pq)

                return run

            def kv_mtile(n):
                def run():
                    cols = slice(n * 512, (n + 1) * 512)
                    pkv = psm.tile([128, 512], F32, tag="m")
                    for kd in range(8):
                        nc.tensor.matmul(
                            pkv,
                            lhsT=wkv_ap(kd),
                            rhs=xt_sb[:, n, kd, :],
                            start=(kd == 0),
                            stop=(kd == 7),
                        )
                    nc.vector.tensor_copy(out=kva_sb[0:64, cols], in_=pkv[0:64, :])
                    nc.vector.tensor_copy(out=vtb_sb[64:128, cols], in_=pkv[64:128, :])
                    # dup kT to partitions 64:128 for the odd-head row group
                    nc.sync.dma_start(out=kvb_sb[64:128, cols], in_=kva_sb[0:64, cols])

                return run

            def v_transpose(n, tt):
                def run():
                    j = n * 4 + tt
                    ptr = psm.tile([128, 64], BF16, tag="m")
                    nc.tensor.transpose(
                        ptr,
                        in_=vtb_sb[64:128, j * 128 : (j + 1) * 128],
                        identity=ident_b[64:128, 64:128],
                    )
                    va = vp.tile([128, 65], BF16, tag="vaug")
                    nc.vector.tensor_copy(out=va[:, 0:64], in_=ptr)
                    nc.gpsimd.memset(va[:, 64:65], 1.0)
                    vaug[j] = va

                return run

            def qkv_pieces(n):
                return [
                    q_mtile(n, 0),
                    q_mtile(n, 1),
                    kv_mtile(n),
                    v_transpose(n, 0),
                    v_transpose(n, 1),
                    v_transpose(n, 2),
                    v_transpose(n, 3),
                ]

            # ---- output projection pieces for chunk c ----
            osb_cur = [None]

            def outproj_tt(c, tt, half=None):
                """half=None: both avt halves; 'a': only avt01 (CAST to osb);
                'b': only avt23 (accumulate onto osb via DVE add)."""

                def run():
                    if tt == 0 and half != "b":
                        osb = op.tile([128, 4, DIM], BF16, tag="osb")
                        osb_cur[0] = osb
                    osb = osb_cur[0]
                    trow = c * 4 + tt
                    tcols = slice(trow * 128, (trow + 1) * 128)
                    for dch in range(2):
                        dcols = slice(dch * 512, (dch + 1) * 512)
                        po = psm.tile([128, 512], F32, tag="m")
                        if half != "b":
                            nc.tensor.matmul(
                                po,
                                lhsT=avt01[:, tcols],
                                rhs=wo_ap(0, dch * 512, (dch + 1) * 512),
                                start=True,
                                stop=(half == "a"),
                            )
                        if half != "a":
                            nc.tensor.matmul(
                                po,
                                lhsT=avt23[:, tcols],
                                rhs=wo_ap(1, dch * 512, (dch + 1) * 512),
                                start=(half == "b"),
                                stop=True,
                            )
                        if half == "b":
                            nc.vector.scalar_tensor_tensor(
                                out=osb[:, tt, dcols],
                                in0=po,
                                scalar=1.0,
                                in1=osb[:, tt, dcols],
                                op0=mybir.AluOpType.mult,
                                op1=mybir.AluOpType.add,
                            )
                        elif dch == 0 or c > 0:
                            nc.vector.tensor_copy(out=osb[:, tt, dcols], in_=po)
                        else:
                            nc.scalar.copy(out=osb[:, tt, dcols], in_=po)
                    if tt == 3 and half != "a":
                        nc.sync.dma_start(
                            out=out[c * 512 : (c + 1) * 512, :].rearrange(
                                "(tt p) n -> p tt n", p=128
                            ),
                            in_=osb,
                        )

                return run

            def outproj_pieces(c):
                return [outproj_tt(c, tt) for tt in range(4)]

            # ---- softmax division + repack for one head pair ----
            def stage(hp, c, av):
                """av: PSUM [65, 2, 512] = (head-in-pair, q)."""
                ccols = slice(c * 512, (c + 1) * 512)
                avt = avt01 if hp == 0 else avt23
                avd = adp.tile([65, 2, 512], BF16, tag="avd")
                nc.vector.tensor_copy(out=avd, in_=av)
                # transpose [65,128] slabs: cols 0:64 av^T, col 64 l^T
                # (66-wide slabs keep PSUM writes 4-byte aligned)
                pt1 = psm.tile([128, 2, 4, 66], BF16, tag="m")
                for i in range(2):
                    for tt in range(4):
                        nc.tensor.matmul(
                            pt1[:, i, tt, 0:65],
                            lhsT=avd[0:65, i, tt * 128 : (tt + 1) * 128],
                            rhs=ident_b[0:65, 0:65],
                            is_transpose=True,
                            start=(i == 0 and tt == 0),
                            stop=(i == 1 and tt == 3),
                        )
                rt = rtp.tile([128, 2, 4], F32, tag="rt")
                nc.vector.reciprocal(out=rt, in_=pt1[:, :, :, 64:65])
                avs = avsp.tile([128, 4, 2, 64], BF16, tag="avs")
                for i in range(2):
                    for tt in range(4):
                        nc.vector.tensor_scalar_mul(
                            out=avs[:, tt, i, :],
                            in0=pt1[:, i, tt, 0:64],
                            scalar1=rt[:, i, tt : tt + 1],
                        )
                # one transpose per tt covers BOTH heads: lhsT free dims
                # (head, feat) flatten to 128 -> out rows 0:64 head-lo,
                # 64:128 head-hi
                pt2 = psm.tile([128, 512], BF16, tag="m")
                for tt in range(4):
                    nc.tensor.matmul(
                        pt2[:, tt * 128 : (tt + 1) * 128],
                        lhsT=avs[:, tt, :, :],
                        rhs=ident_b,
                        is_transpose=True,
                        start=(tt == 0),
                        stop=(tt == 3),
                    )
                # head-lo -> partitions 0:64 directly; head-hi via DMA bounce
                nc.vector.tensor_copy(out=avt[0:64, ccols], in_=pt2[0:64, :])
                avh = ahp.tile([128, 512], BF16, tag="avh")
                nc.vector.tensor_copy(out=avh[64:128, :], in_=pt2[64:128, :])
                nc.sync.dma_start(out=avt[64:128, ccols], in_=avh[64:128, :])

            # ---- attention for one chunk ----
            def attn_c(c, bg, bg1=None):
                njc = 4 * c + 4

                def av_mms(av, j, njc):
                    sa, w, _ = _cfg(c, j)
                    ptt = ptts[j]
                    st, sp = (j == 0), (j == njc - 1)
                    for i in range(2):
                        nc.tensor.matmul(
                            av[:, i, sa : sa + w],
                            lhsT=vaug[j],
                            rhs=ptt[:, i, 0:w],
                            start=st,
                            stop=sp,
                        )

                for hp in range(2):
                    if hp == 1 and bg1:
                        bg = bg1 + bg
                    av = psav.tile([65, 2, 512], F32, tag="av")
                    ptts = [None] * njc
                    for j in range(njc):
                        sa, w, diag = _cfg(c, j)
                        jk = slice(j * 128, (j + 1) * 128)
                        qc = slice(c * 512 + sa, c * 512 + sa + w)
                        spt = pss.tile([128, 2, 512], F32, tag="s")
                        # paired score matmuls on row groups 0 / 64 run
                        # concurrently when they stay adjacent on the PE
                        nc.tensor.matmul(
                            spt[:, 0, 0:w],
                            lhsT=kva_sb[0:64, jk],
                            rhs=qt_sb[0:64, hp, qc],
                            start=True,
                            stop=not diag,
                        )
                        nc.tensor.matmul(
                            spt[:, 1, 0:w],
                            lhsT=kvb_sb[64:128, jk],
                            rhs=qt_sb[64:128, hp, qc],
                            start=True,
                            stop=not diag,
                        )
                        if diag:
                            for i in range(2):
                                nc.tensor.matmul(
                                    spt[0:64, i, 0:128],
                                    lhsT=ident_b[0:64, 0:64],
                                    rhs=mlt_sb[0:64, 0:128],
                                    start=False,
                                    stop=False,
                                )
                                nc.tensor.matmul(
                                    spt[64:128, i, 0:128],
                                    lhsT=ident_b[64:128, 64:128],
                                    rhs=mlt_sb[64:128, 0:128],
                                    start=False,
                                    stop=True,
                                )
                        ptt = ptp.tile([128, 2, 512], BF16, tag="ptt")
                        ptts[j] = ptt
                        nc.scalar.activation(
                            out=ptt[:, :, 0:w],
                            in_=spt[:, :, 0:w],
                            func=mybir.ActivationFunctionType.Exp,
                            scale=0.125,
                        )
                        # software pipeline: av for the PREVIOUS tile (its
                        # exp is done) keeps the PE FIFO off this tile's exp
                        if j > 0:
                            av_mms(av, j - 1, njc)
                        if bg:
                            bg.pop(0)()
                    av_mms(av, njc - 1, njc)
                    stage(hp, c, av)

            # ---- schedule: qkv(c+1) + outproj(c-1) interleave into attn(c)
            for piece in qkv_pieces(0):
                piece()
            for c in range(NCH):
                bg = []
                if c + 1 < NCH:
                    bg += qkv_pieces(c + 1)
                if c >= 1:
                    bg += outproj_pieces(c - 1)
                bg1 = None
                if c == NCH - 1:
                    # avt01-half of the last out-projection can run as soon
                    # as stage(hp0) lands -- schedule it into hp1's bg slots
                    bg1 = [outproj_tt(c, tt, half="a") for tt in range(4)]
                attn_c(c, bg, bg1)
                for piece in bg:  # leftovers (bg longer than j iters)
                    piece()
            for tt in range(4):
                outproj_tt(NCH - 1, tt, half="b")()

    nc.compile()
    return nc


def _mask_lt():
    idx = np.arange(128)
    return np.where(idx[:, None] > idx[None, :], NEG, 0.0).astype(ml_dtypes.bfloat16)


def make_in_maps(x, Wq, Wkv, Wo):
    x = np.asarray(x, dtype=np.float32)
    Wq = np.asarray(Wq, dtype=np.float32)
    Wkv = np.asarray(Wkv, dtype=np.float32)
    Wo = np.asarray(Wo, dtype=np.float32)
    in_maps = []
    for core in range(8):
        b, g = divmod(core, NKV)
        k_loc = Wkv[:, g * HD : (g + 1) * HD]
        v_loc = Wkv[:, NKV * HD + g * HD : NKV * HD + (g + 1) * HD]
        # xp[p, n*4096 + kd*512 + t] = x[b][n*512 + t, kd*128 + p]
        xpk = (
            x[b]
            .T.reshape(8, 128, NCH, 512)
            .transpose(1, 2, 0, 3)
            .reshape(128, NCH * 4096)
            .astype(ml_dtypes.bfloat16)
        )
        wq_p = (
            Wq[:, g * HQ : (g + 1) * HQ]
            .reshape(8, 128, HQ)
            .transpose(1, 0, 2)
            .reshape(128, 2048)
        )
        wkv_p = (
            np.concatenate([k_loc, v_loc], axis=1)
            .reshape(8, 128, 128)
            .transpose(1, 0, 2)
            .reshape(128, 1024)
        )
        w1 = np.concatenate([wq_p, wkv_p], axis=1).astype(ml_dtypes.bfloat16)
        wo_p = (
            Wo[g * HQ : (g + 1) * HQ, :]
            .reshape(2, 128, DIM)
            .transpose(1, 0, 2)
            .reshape(128, 2 * DIM)
        )
        w2 = np.concatenate(
            [np.eye(128, dtype=np.float32), _mask_lt().astype(np.float32), wo_p],
            axis=1,
        ).astype(ml_dtypes.bfloat16)
        in_maps.append(
            {
                "xp": np.ascontiguousarray(xpk),
                "w1": np.ascontiguousarray(w1),
                "w2": np.ascontiguousarray(w2),
            }
        )
    return in_maps


def gather(results):
    outs = [results[i]["out"].astype(np.float64) for i in range(8)]
    return np.stack(
        [
            outs[0] + outs[1] + outs[2] + outs[3],
            outs[4] + outs[5] + outs[6] + outs[7],
        ]
    ).astype(np.float32)


def kernel(x, Wq, Wkv, Wo):
    global _CACHED_NC
    if _CACHED_NC is None:
        _CACHED_NC = build_nc()
    in_maps = make_in_maps(x, Wq, Wkv, Wo)
    res = run_bass_kernel_spmd(_CACHED_NC, in_maps, list(range(8)))
    return gather(res.results)
